# revision 11
# baseline (speedup 1.0000x reference)
"""Trainium2 Bass kernel v4: 16-filter binarized 5x5 VALID conv.

x [32, 6, 512, 512] f32 -> out [32, 16, 508, 508] f32, data-parallel over
batch on 8 cores (4 images/core).

Per-core mapping, designed around DMA *packet* throughput (the measured
wall): every HBM descriptor is multi-KB.

  Row-pair groups: each main group covers 16 output rows (31 groups +
  one 12-row tail per image). Contraction K = 114 = (channel c in 6) x
  (pair-base j in 19); SBUF partition (c,j) holds TWO consecutive image
  rows (16G+j, 16G+j+1) contiguously -> load descriptors are 2 KB.
  M = 128 PSUM partitions = (o in 16) x (row-pair rb in 8); the moving
  operand streams N = 1016 = (rp in 2) x (n in 508) via a 2D AP
  [[512,2],[1,508]], so PSUM partition m=(o,rb) accumulates output rows
  16G+2rb and 16G+2rb+1 side by side (psum [128, 1016], two banks).
  Five accumulating matmuls per group (dx = moving-base offset 0..4).

  Stores write PSUM-native layout out_main [b, 128, 31*1016] f32
  (descriptors 16 KB, contiguous per partition); the host un-permutes.
  Weights are exact +/-1 signs in bf16; the per-filter alpha scale folds
  into the PSUM evacuation (per-partition multiply), alternating
  DVE / ACT engines so neither is the bottleneck.
"""

import numpy as np
import ml_dtypes

import concourse.bass as bass
import concourse.mybir as mybir
from concourse import bacc
from concourse import tile
from concourse.bass_utils import run_bass_kernel_spmd

MAPS3 = np.array([[0, 1, 2], [1, 2, 3], [2, 3, 4], [3, 4, 5], [0, 4, 5], [0, 1, 5]])
MAPS4 = np.array(
    [
        [0, 1, 2, 3],
        [1, 2, 3, 4],
        [2, 3, 4, 5],
        [0, 3, 4, 5],
        [0, 1, 4, 5],
        [0, 1, 2, 5],
        [0, 1, 3, 4],
        [1, 2, 4, 5],
        [0, 2, 3, 5],
    ]
)

C_IN = 6
N_OUT = 16
KH = KW = 5
RG = 16  # output rows per main group
NJ = 19  # pair-bases per channel (rows 0..19 of the 20-row window)
K_MAIN = C_IN * NJ  # 114
NRB = 8  # row-pairs per group
M_MAIN = N_OUT * NRB  # 128
R_TAIL = 12  # output rows in the tail group (496..507)
NJ_T = 15  # tail pair-bases (window rows 0..15)
K_TAIL = C_IN * NJ_T  # 90 (padded to 96 partitions)
K_TAIL_P = 96
NRB_T = 6
M_TAIL = N_OUT * NRB_T  # 96
N_CORES = 8
NG = 8  # groups per chunk tile (chunks: 8,8,8,7)
SG = 8  # groups per staged store


def _binarize_np(w):
    w = np.asarray(w, dtype=np.float32)
    m = w - w.mean(axis=1, keepdims=True)
    c = np.clip(m, -1.0, 1.0)
    alpha = np.abs(c).mean(axis=(1, 2, 3))
    return np.sign(c).astype(np.float32), alpha.astype(np.float32)


def _filter_table(w3, w4, w6):
    s3, a3 = _binarize_np(w3)
    s4, a4 = _binarize_np(w4)
    s6, a6 = _binarize_np(w6)
    table = []
    for o in range(6):
        table.append((list(MAPS3[o]), s3[o], a3[o]))
    for o in range(9):
        table.append((list(MAPS4[o]), s4[o], a4[o]))
    table.append((list(range(6)), s6[0], a6[0]))
    return table


def _build_weight_inputs(w3, w4, w6):
    """wm [114, 5*128], wmt [96, 5*96] bf16; al [128,1], alt [96,1] f32."""
    table = _filter_table(w3, w4, w6)
    wm = np.zeros((K_MAIN, KW * M_MAIN), dtype=np.float32)
    wmt = np.zeros((K_TAIL_P, KW * M_TAIL), dtype=np.float32)
    al = np.zeros((M_MAIN, 1), dtype=np.float32)
    alt = np.zeros((M_TAIL, 1), dtype=np.float32)
    for o, (chans, sgn, alpha) in enumerate(table):
        for rb in range(NRB):
            m = o * NRB + rb
            al[m, 0] = alpha
            for dx in range(KW):
                for ci, c in enumerate(chans):
                    for dy in range(KH):
                        # partition order (j, c): j-outer so each load DMA
                        # spreads 19 units across all 16 SDMA engines
                        wm[(2 * rb + dy) * C_IN + c, dx * M_MAIN + m] = sgn[
                            ci, dy, dx
                        ]
        for rb in range(NRB_T):
            m = o * NRB_T + rb
            alt[m, 0] = alpha
            for dx in range(KW):
                for ci, c in enumerate(chans):
                    for dy in range(KH):
                        wmt[(2 * rb + dy) * C_IN + c, dx * M_TAIL + m] = sgn[
                            ci, dy, dx
                        ]
    return (
        wm.astype(ml_dtypes.bfloat16),
        wmt.astype(ml_dtypes.bfloat16),
        al,
        alt,
    )


def build_nc(b_per_core, h, w, num_cores=N_CORES):
    h_out, w_out = h - KH + 1, w - KW + 1
    n_groups = (h_out - R_TAIL) // RG  # 31
    assert n_groups * RG + R_TAIL == h_out
    tail_start = h_out - R_TAIL  # 496
    NPAIR = 2 * w  # elems per pair-slot (1024)
    NN = 2 * w_out  # matmul N (1016)
    f32 = mybir.dt.float32
    bf16 = mybir.dt.bfloat16

    chunks = []
    g0 = 0
    while g0 < n_groups:
        chunks.append((g0, min(NG, n_groups - g0)))
        g0 += NG

    nc = bacc.Bacc(
        "TRN2",
        target_bir_lowering=False,
        debug=False,
        num_devices=num_cores,
    )
    x_t = nc.dram_tensor("xb", [b_per_core, C_IN, h, w], bf16, kind="ExternalInput")
    wm_t = nc.dram_tensor("wm", [K_MAIN, KW * M_MAIN], bf16, kind="ExternalInput")
    wmt_t = nc.dram_tensor("wmt", [K_TAIL_P, KW * M_TAIL], bf16, kind="ExternalInput")
    al_t = nc.dram_tensor("al", [M_MAIN, 1], f32, kind="ExternalInput")
    alt_t = nc.dram_tensor("alt", [M_TAIL, 1], f32, kind="ExternalInput")
    om_t = nc.dram_tensor(
        "out_main", [b_per_core, M_MAIN, n_groups * NN], bf16, kind="ExternalOutput"
    )
    ot_t = nc.dram_tensor(
        "out_tail", [b_per_core, M_TAIL, NN], bf16, kind="ExternalOutput"
    )

    with tile.TileContext(nc) as tc:
        with (
            tc.tile_pool(name="wpool", bufs=1) as wpool,
            tc.tile_pool(name="xpool", bufs=5) as xpool,
            tc.tile_pool(name="tpool", bufs=2) as tpool,
            tc.tile_pool(name="spool", bufs=3) as spool,
            tc.tile_pool(name="s2pool", bufs=2) as s2pool,
            tc.tile_pool(name="ppool", bufs=3, space="PSUM") as ppool,
            tc.tile_pool(name="p2pool", bufs=1, space="PSUM") as p2pool,
        ):
            wt = wpool.tile([K_MAIN, KW * M_MAIN], bf16, tag="wt")
            nc.sync.dma_start(out=wt[:], in_=wm_t[:])
            wtt = wpool.tile([K_TAIL_P, KW * M_TAIL], bf16, tag="wtt")
            nc.sync.dma_start(out=wtt[:], in_=wmt_t[:])
            at = wpool.tile([M_MAIN, 1], f32, tag="at")
            nc.sync.dma_start(out=at[:], in_=al_t[:])
            att = wpool.tile([M_TAIL, 1], f32, tag="att")
            nc.sync.dma_start(out=att[:], in_=alt_t[:])

            def load_chunk(b, ci):
                g0, ng = chunks[ci]
                xt = xpool.tile(
                    [K_MAIN, NG * NPAIR], bf16, tag="xt", name=f"xt_{b}_{ci}"
                )
                # one DMA per group-slot: 114 partitions (2-level partition
                # AP over c,j) -> all 16 SDMA engines, ~1.5us drain each
                for gl in range(ng):
                    src = bass.AP(
                        x_t,
                        b * C_IN * h * w + RG * (g0 + gl) * w,
                        [[w, NJ], [h * w, C_IN], [1, NPAIR]],
                    )
                    eng = nc.sync if gl % 2 == 0 else nc.scalar
                    eng.dma_start(
                        out=xt[:, gl * NPAIR : (gl + 1) * NPAIR], in_=src
                    )
                return xt

            def load_tail(b):
                xt2 = tpool.tile([K_TAIL_P, NPAIR], bf16, tag="xtt", name=f"xtt_{b}")
                src = bass.AP(
                    x_t,
                    b * C_IN * h * w + tail_start * w,
                    [[w, NJ_T], [h * w, C_IN], [1, NPAIR]],
                )
                nc.sync.dma_start(out=xt2[0:K_TAIL, :], in_=src)
                # pad partitions 90..95: any finite data (zero weights)
                psrc = bass.AP(x_t, b * C_IN * h * w, [[w, 6], [1, NPAIR]])
                nc.sync.dma_start(out=xt2[K_TAIL:K_TAIL_P, :], in_=psrc)
                return xt2

            def do_group(xt, gl, evac_dve, stg, soff):
                pss = [
                    ppool.tile([M_MAIN, w_out], f32, tag=f"ps{rp}", name=f"ps{rp}")
                    for rp in range(2)
                ]
                # dx-outer so each LDWEIGHTS serves both rp matmuls
                for dx in range(KW):
                    for rp in range(2):
                        rhs = bass.AP(
                            xt[:].tensor,
                            gl * NPAIR + rp * w + dx,
                            [[NG * NPAIR, K_MAIN], [1, w_out]],
                        )
                        nc.tensor.matmul(
                            pss[rp][:],
                            wt[:, dx * M_MAIN : (dx + 1) * M_MAIN],
                            rhs,
                            start=(dx == 0),
                            stop=(dx == KW - 1),
                        )
                for rp in range(2):
                    dst = stg[:, soff + rp * w_out : soff + (rp + 1) * w_out]
                    if evac_dve:
                        nc.vector.tensor_scalar_mul(dst, pss[rp][:], at[:])
                    else:
                        nc.scalar.mul(dst, pss[rp][:], at[:])

            def store_stage(b, gs, stg, ng_st):
                dst = bass.AP(
                    om_t,
                    b * M_MAIN * n_groups * NN + gs * NN,
                    [[n_groups * NN, M_MAIN], [1, ng_st * NN]],
                )
                nc.scalar.dma_start(out=dst, in_=stg[:, : ng_st * NN])

            def do_tail(b, xt2):
                pss = [
                    p2pool.tile([M_TAIL, w_out], f32, tag=f"pst{rp}", name=f"pst{rp}")
                    for rp in range(2)
                ]
                for dx in range(KW):
                    for rp in range(2):
                        rhs = bass.AP(
                            xt2[:].tensor,
                            rp * w + dx,
                            [[NPAIR, K_TAIL_P], [1, w_out]],
                        )
                        nc.tensor.matmul(
                            pss[rp][:],
                            wtt[:, dx * M_TAIL : (dx + 1) * M_TAIL],
                            rhs,
                            start=(dx == 0),
                            stop=(dx == KW - 1),
                        )
                st = s2pool.tile([M_TAIL, NN], bf16, tag="st2")
                for rp in range(2):
                    nc.vector.tensor_scalar_mul(
                        st[:, rp * w_out : (rp + 1) * w_out], pss[rp][:], att[:]
                    )
                dst = bass.AP(
                    ot_t, b * M_TAIL * NN, [[NN, M_TAIL], [1, NN]]
                )
                nc.scalar.dma_start(out=dst, in_=st[:])

            units = [(b, ci) for b in range(b_per_core) for ci in range(len(chunks))]
            LOOKAHEAD = 4
            xtiles = {}
            ttiles = {}
            for u in range(min(LOOKAHEAD, len(units))):
                b, ci = units[u]
                xtiles[(b, ci)] = load_chunk(b, ci)
                if ci == 0:
                    ttiles[b] = load_tail(b)

            for u, (b, ci) in enumerate(units):
                ul = u + LOOKAHEAD
                if ul < len(units):
                    bl, cl = units[ul]
                    xtiles[(bl, cl)] = load_chunk(bl, cl)
                    if cl == 0:
                        ttiles[bl] = load_tail(bl)
                xt = xtiles.pop((b, ci))
                g0, ng = chunks[ci]
                gl = 0
                while gl < ng:
                    ns = min(SG, ng - gl)
                    stg = spool.tile(
                        [M_MAIN, SG * NN], bf16, tag="stg",
                        name=f"stg_{b}_{ci}_{gl}",
                    )
                    for gg in range(ns):
                        do_group(xt, gl + gg, (gg % 2 == 0), stg, gg * NN)
                    store_stage(b, g0 + gl, stg, ns)
                    gl += ns
                if ci == len(chunks) - 1:
                    do_tail(b, ttiles.pop(b))

    nc.compile()
    return nc


_NC_CACHE = {}


def _get_nc(b_per_core, h, w):
    key = (b_per_core, h, w)
    if key not in _NC_CACHE:
        _NC_CACHE[key] = build_nc(b_per_core, h, w)
    return _NC_CACHE[key]


def _prep_inputs(x, w3, w4, w6):
    b = x.shape[0]
    assert b % N_CORES == 0
    bpc = b // N_CORES
    wm, wmt, al, alt = _build_weight_inputs(w3, w4, w6)
    xb = np.ascontiguousarray(x).astype(ml_dtypes.bfloat16)
    in_maps = [
        {
            "xb": np.ascontiguousarray(xb[i * bpc : (i + 1) * bpc]),
            "wm": wm,
            "wmt": wmt,
            "al": al,
            "alt": alt,
        }
        for i in range(N_CORES)
    ]
    return bpc, in_maps


def _unpermute(om, ot, bpc, h_out, w_out):
    """om [bpc, 128, 31*1016], ot [bpc, 96, 1016] -> [bpc, 16, 508, 508]."""
    n_groups = (h_out - R_TAIL) // RG
    out = np.empty((bpc, N_OUT, h_out, w_out), dtype=np.float32)
    m = om.reshape(bpc, N_OUT, NRB, n_groups, 2, w_out)
    out[:, :, : n_groups * RG] = m.transpose(0, 1, 3, 2, 4, 5).reshape(
        bpc, N_OUT, n_groups * RG, w_out
    )
    t = ot.reshape(bpc, N_OUT, NRB_T * 2, w_out)
    out[:, :, n_groups * RG :] = t
    return out


def run(x, w3, w4, w6, trace=False, **kw):
    b, c, h, w = x.shape
    h_out, w_out = h - 4, w - 4
    bpc, in_maps = _prep_inputs(x, w3, w4, w6)
    nc = _get_nc(bpc, h, w)
    res = run_bass_kernel_spmd(
        nc, in_maps, list(range(N_CORES)), trace=trace, **kw
    )
    outs = [
        _unpermute(
            np.asarray(r["out_main"], dtype=np.float32),
            np.asarray(r["out_tail"], dtype=np.float32),
            bpc, h_out, w_out,
        )
        for r in res.results
    ]
    return np.concatenate(outs, axis=0), res


def kernel(x, w3, w4, w6):
    out, _ = run(x, w3, w4, w6, trace=False)
    return out



# revision 12
# speedup vs baseline: 3.5751x; 3.5751x over previous
"""Trainium2 Bass kernel v4: 16-filter binarized 5x5 VALID conv.

x [32, 6, 512, 512] f32 -> out [32, 16, 508, 508] f32, data-parallel over
batch on 8 cores (4 images/core).

Per-core mapping, designed around DMA *packet* throughput (the measured
wall): every HBM descriptor is multi-KB.

  Row-pair groups: each main group covers 16 output rows (31 groups +
  one 12-row tail per image). Contraction K = 114 = (channel c in 6) x
  (pair-base j in 19); SBUF partition (c,j) holds TWO consecutive image
  rows (16G+j, 16G+j+1) contiguously -> load descriptors are 2 KB.
  M = 128 PSUM partitions = (o in 16) x (row-pair rb in 8); the moving
  operand streams N = 1016 = (rp in 2) x (n in 508) via a 2D AP
  [[512,2],[1,508]], so PSUM partition m=(o,rb) accumulates output rows
  16G+2rb and 16G+2rb+1 side by side (psum [128, 1016], two banks).
  Five accumulating matmuls per group (dx = moving-base offset 0..4).

  Stores write PSUM-native layout out_main [b, 128, 31*1016] f32
  (descriptors 16 KB, contiguous per partition); the host un-permutes.
  Weights are exact +/-1 signs in bf16; the per-filter alpha scale folds
  into the PSUM evacuation (per-partition multiply), alternating
  DVE / ACT engines so neither is the bottleneck.
"""

import numpy as np
import ml_dtypes

import concourse.bass as bass
import concourse.mybir as mybir
from concourse import bacc
from concourse import tile
from concourse.bass_utils import run_bass_kernel_spmd

MAPS3 = np.array([[0, 1, 2], [1, 2, 3], [2, 3, 4], [3, 4, 5], [0, 4, 5], [0, 1, 5]])
MAPS4 = np.array(
    [
        [0, 1, 2, 3],
        [1, 2, 3, 4],
        [2, 3, 4, 5],
        [0, 3, 4, 5],
        [0, 1, 4, 5],
        [0, 1, 2, 5],
        [0, 1, 3, 4],
        [1, 2, 4, 5],
        [0, 2, 3, 5],
    ]
)

C_IN = 6
N_OUT = 16
KH = KW = 5
RG = 16  # output rows per main group
NJ = 19  # pair-bases per channel (rows 0..19 of the 20-row window)
K_MAIN = C_IN * NJ  # 114
NRB = 8  # row-pairs per group
M_MAIN = N_OUT * NRB  # 128
R_TAIL = 12  # output rows in the tail group (496..507)
NJ_T = 15  # tail pair-bases (window rows 0..15)
K_TAIL = C_IN * NJ_T  # 90 (padded to 96 partitions)
K_TAIL_P = 96
NRB_T = 6
M_TAIL = N_OUT * NRB_T  # 96
N_CORES = 8
NG = 8  # groups per chunk tile (chunks: 8,8,8,7)
SG = 8  # groups per staged store


def _binarize_np(w):
    w = np.asarray(w, dtype=np.float32)
    m = w - w.mean(axis=1, keepdims=True)
    c = np.clip(m, -1.0, 1.0)
    alpha = np.abs(c).mean(axis=(1, 2, 3))
    return np.sign(c).astype(np.float32), alpha.astype(np.float32)


def _filter_table(w3, w4, w6):
    s3, a3 = _binarize_np(w3)
    s4, a4 = _binarize_np(w4)
    s6, a6 = _binarize_np(w6)
    table = []
    for o in range(6):
        table.append((list(MAPS3[o]), s3[o], a3[o]))
    for o in range(9):
        table.append((list(MAPS4[o]), s4[o], a4[o]))
    table.append((list(range(6)), s6[0], a6[0]))
    return table


def _build_weight_inputs(w3, w4, w6):
    """wm [114, 5*128], wmt [96, 5*96] bf16; al [128,1], alt [96,1] f32."""
    table = _filter_table(w3, w4, w6)
    wm = np.zeros((K_MAIN, KW * M_MAIN), dtype=np.float32)
    wmt = np.zeros((K_TAIL_P, KW * M_TAIL), dtype=np.float32)
    al = np.zeros((M_MAIN, 1), dtype=np.float32)
    alt = np.zeros((M_TAIL, 1), dtype=np.float32)
    for o, (chans, sgn, alpha) in enumerate(table):
        for rb in range(NRB):
            m = o * NRB + rb
            al[m, 0] = alpha
            for dx in range(KW):
                for ci, c in enumerate(chans):
                    for dy in range(KH):
                        # partition order (j, c): j-outer so each load DMA
                        # spreads 19 units across all 16 SDMA engines
                        wm[(2 * rb + dy) * C_IN + c, dx * M_MAIN + m] = sgn[
                            ci, dy, dx
                        ]
        for rb in range(NRB_T):
            m = o * NRB_T + rb
            alt[m, 0] = alpha
            for dx in range(KW):
                for ci, c in enumerate(chans):
                    for dy in range(KH):
                        wmt[(2 * rb + dy) * C_IN + c, dx * M_TAIL + m] = sgn[
                            ci, dy, dx
                        ]
    return (
        wm.astype(ml_dtypes.bfloat16),
        wmt.astype(ml_dtypes.bfloat16),
        al,
        alt,
    )


def build_nc(b_per_core, h, w, num_cores=N_CORES):
    h_out, w_out = h - KH + 1, w - KW + 1
    n_groups = (h_out - R_TAIL) // RG  # 31
    assert n_groups * RG + R_TAIL == h_out
    tail_start = h_out - R_TAIL  # 496
    NPAIR = 2 * w  # elems per pair-slot (1024)
    NN = 2 * w_out  # matmul N (1016)
    f32 = mybir.dt.float32
    bf16 = mybir.dt.bfloat16

    chunks = []
    g0 = 0
    while g0 < n_groups:
        chunks.append((g0, min(NG, n_groups - g0)))
        g0 += NG

    nc = bacc.Bacc(
        "TRN2",
        target_bir_lowering=False,
        debug=False,
        num_devices=num_cores,
    )
    x_t = nc.dram_tensor("xb", [b_per_core, C_IN, h, w], bf16, kind="ExternalInput")
    wm_t = nc.dram_tensor("wm", [K_MAIN, KW * M_MAIN], bf16, kind="ExternalInput")
    wmt_t = nc.dram_tensor("wmt", [K_TAIL_P, KW * M_TAIL], bf16, kind="ExternalInput")
    al_t = nc.dram_tensor("al", [M_MAIN, 1], f32, kind="ExternalInput")
    alt_t = nc.dram_tensor("alt", [M_TAIL, 1], f32, kind="ExternalInput")
    om_t = nc.dram_tensor(
        "out_main", [b_per_core, M_MAIN, n_groups * NN], bf16, kind="ExternalOutput"
    )
    ot_t = nc.dram_tensor(
        "out_tail", [b_per_core, M_TAIL, NN], bf16, kind="ExternalOutput"
    )

    with tile.TileContext(nc) as tc:
        with (
            tc.tile_pool(name="wpool", bufs=1) as wpool,
            tc.tile_pool(name="xpool", bufs=5) as xpool,
            tc.tile_pool(name="tpool", bufs=2) as tpool,
            tc.tile_pool(name="spool", bufs=3) as spool,
            tc.tile_pool(name="s2pool", bufs=2) as s2pool,
            tc.tile_pool(name="ppool", bufs=3, space="PSUM") as ppool,
            tc.tile_pool(name="p2pool", bufs=1, space="PSUM") as p2pool,
        ):
            wt = wpool.tile([K_MAIN, KW * M_MAIN], bf16, tag="wt")
            nc.sync.dma_start(out=wt[:], in_=wm_t[:])
            wtt = wpool.tile([K_TAIL_P, KW * M_TAIL], bf16, tag="wtt")
            nc.sync.dma_start(out=wtt[:], in_=wmt_t[:])
            at = wpool.tile([M_MAIN, 1], f32, tag="at")
            nc.sync.dma_start(out=at[:], in_=al_t[:])
            att = wpool.tile([M_TAIL, 1], f32, tag="att")
            nc.sync.dma_start(out=att[:], in_=alt_t[:])

            def load_chunk(b, ci):
                g0, ng = chunks[ci]
                xt = xpool.tile(
                    [K_MAIN, NG * NPAIR], bf16, tag="xt", name=f"xt_{b}_{ci}"
                )
                # one DMA per group-slot: 114 partitions (2-level partition
                # AP over c,j) -> all 16 SDMA engines, ~1.5us drain each
                # dim0 of a 3D DMA AP maps 1:1 onto SDMA engines and must be
                # <=16 to fan out; split j 19 -> 16 + 3
                for gl in range(ng):
                    base = b * C_IN * h * w + RG * (g0 + gl) * w
                    eng = nc.sync if gl % 2 == 0 else nc.scalar
                    src_a = bass.AP(
                        x_t, base, [[w, 16], [h * w, C_IN], [1, NPAIR]]
                    )
                    eng.dma_start(
                        out=xt[0 : 16 * C_IN, gl * NPAIR : (gl + 1) * NPAIR],
                        in_=src_a,
                    )
                    src_b = bass.AP(
                        x_t, base + 16 * w, [[w, 3], [h * w, C_IN], [1, NPAIR]]
                    )
                    eng.dma_start(
                        out=xt[16 * C_IN : K_MAIN, gl * NPAIR : (gl + 1) * NPAIR],
                        in_=src_b,
                    )
                return xt

            def load_tail(b):
                xt2 = tpool.tile([K_TAIL_P, NPAIR], bf16, tag="xtt", name=f"xtt_{b}")
                src = bass.AP(
                    x_t,
                    b * C_IN * h * w + tail_start * w,
                    [[w, NJ_T], [h * w, C_IN], [1, NPAIR]],
                )
                nc.sync.dma_start(out=xt2[0:K_TAIL, :], in_=src)
                # pad partitions 90..95: any finite data (zero weights)
                psrc = bass.AP(x_t, b * C_IN * h * w, [[w, 6], [1, NPAIR]])
                nc.sync.dma_start(out=xt2[K_TAIL:K_TAIL_P, :], in_=psrc)
                return xt2

            def do_group(xt, gl, evac_dve, stg, soff):
                pss = [
                    ppool.tile([M_MAIN, w_out], f32, tag=f"ps{rp}", name=f"ps{rp}")
                    for rp in range(2)
                ]
                # dx-outer so each LDWEIGHTS serves both rp matmuls
                for dx in range(KW):
                    for rp in range(2):
                        rhs = bass.AP(
                            xt[:].tensor,
                            gl * NPAIR + rp * w + dx,
                            [[NG * NPAIR, K_MAIN], [1, w_out]],
                        )
                        nc.tensor.matmul(
                            pss[rp][:],
                            wt[:, dx * M_MAIN : (dx + 1) * M_MAIN],
                            rhs,
                            start=(dx == 0),
                            stop=(dx == KW - 1),
                        )
                for rp in range(2):
                    dst = stg[:, soff + rp * w_out : soff + (rp + 1) * w_out]
                    if evac_dve:
                        nc.vector.tensor_scalar_mul(dst, pss[rp][:], at[:])
                    else:
                        nc.scalar.mul(dst, pss[rp][:], at[:])

            def store_stage(b, gs, stg, ng_st):
                dst = bass.AP(
                    om_t,
                    b * M_MAIN * n_groups * NN + gs * NN,
                    [[n_groups * NN, M_MAIN], [1, ng_st * NN]],
                )
                nc.scalar.dma_start(out=dst, in_=stg[:, : ng_st * NN])

            def do_tail(b, xt2):
                pss = [
                    p2pool.tile([M_TAIL, w_out], f32, tag=f"pst{rp}", name=f"pst{rp}")
                    for rp in range(2)
                ]
                for dx in range(KW):
                    for rp in range(2):
                        rhs = bass.AP(
                            xt2[:].tensor,
                            rp * w + dx,
                            [[NPAIR, K_TAIL_P], [1, w_out]],
                        )
                        nc.tensor.matmul(
                            pss[rp][:],
                            wtt[:, dx * M_TAIL : (dx + 1) * M_TAIL],
                            rhs,
                            start=(dx == 0),
                            stop=(dx == KW - 1),
                        )
                st = s2pool.tile([M_TAIL, NN], bf16, tag="st2")
                for rp in range(2):
                    nc.vector.tensor_scalar_mul(
                        st[:, rp * w_out : (rp + 1) * w_out], pss[rp][:], att[:]
                    )
                dst = bass.AP(
                    ot_t, b * M_TAIL * NN, [[NN, M_TAIL], [1, NN]]
                )
                nc.scalar.dma_start(out=dst, in_=st[:])

            units = [(b, ci) for b in range(b_per_core) for ci in range(len(chunks))]
            LOOKAHEAD = 4
            xtiles = {}
            ttiles = {}
            for u in range(min(LOOKAHEAD, len(units))):
                b, ci = units[u]
                xtiles[(b, ci)] = load_chunk(b, ci)
                if ci == 0:
                    ttiles[b] = load_tail(b)

            for u, (b, ci) in enumerate(units):
                ul = u + LOOKAHEAD
                if ul < len(units):
                    bl, cl = units[ul]
                    xtiles[(bl, cl)] = load_chunk(bl, cl)
                    if cl == 0:
                        ttiles[bl] = load_tail(bl)
                xt = xtiles.pop((b, ci))
                g0, ng = chunks[ci]
                gl = 0
                while gl < ng:
                    ns = min(SG, ng - gl)
                    stg = spool.tile(
                        [M_MAIN, SG * NN], bf16, tag="stg",
                        name=f"stg_{b}_{ci}_{gl}",
                    )
                    for gg in range(ns):
                        do_group(xt, gl + gg, (gg % 2 == 0), stg, gg * NN)
                    store_stage(b, g0 + gl, stg, ns)
                    gl += ns
                if ci == len(chunks) - 1:
                    do_tail(b, ttiles.pop(b))

    nc.compile()
    return nc


_NC_CACHE = {}


def _get_nc(b_per_core, h, w):
    key = (b_per_core, h, w)
    if key not in _NC_CACHE:
        _NC_CACHE[key] = build_nc(b_per_core, h, w)
    return _NC_CACHE[key]


def _prep_inputs(x, w3, w4, w6):
    b = x.shape[0]
    assert b % N_CORES == 0
    bpc = b // N_CORES
    wm, wmt, al, alt = _build_weight_inputs(w3, w4, w6)
    xb = np.ascontiguousarray(x).astype(ml_dtypes.bfloat16)
    in_maps = [
        {
            "xb": np.ascontiguousarray(xb[i * bpc : (i + 1) * bpc]),
            "wm": wm,
            "wmt": wmt,
            "al": al,
            "alt": alt,
        }
        for i in range(N_CORES)
    ]
    return bpc, in_maps


def _unpermute(om, ot, bpc, h_out, w_out):
    """om [bpc, 128, 31*1016], ot [bpc, 96, 1016] -> [bpc, 16, 508, 508]."""
    n_groups = (h_out - R_TAIL) // RG
    out = np.empty((bpc, N_OUT, h_out, w_out), dtype=np.float32)
    m = om.reshape(bpc, N_OUT, NRB, n_groups, 2, w_out)
    out[:, :, : n_groups * RG] = m.transpose(0, 1, 3, 2, 4, 5).reshape(
        bpc, N_OUT, n_groups * RG, w_out
    )
    t = ot.reshape(bpc, N_OUT, NRB_T * 2, w_out)
    out[:, :, n_groups * RG :] = t
    return out


def run(x, w3, w4, w6, trace=False, **kw):
    b, c, h, w = x.shape
    h_out, w_out = h - 4, w - 4
    bpc, in_maps = _prep_inputs(x, w3, w4, w6)
    nc = _get_nc(bpc, h, w)
    res = run_bass_kernel_spmd(
        nc, in_maps, list(range(N_CORES)), trace=trace, **kw
    )
    outs = [
        _unpermute(
            np.asarray(r["out_main"], dtype=np.float32),
            np.asarray(r["out_tail"], dtype=np.float32),
            bpc, h_out, w_out,
        )
        for r in res.results
    ]
    return np.concatenate(outs, axis=0), res


def kernel(x, w3, w4, w6):
    out, _ = run(x, w3, w4, w6, trace=False)
    return out



# revision 13
# speedup vs baseline: 4.2030x; 1.1756x over previous
"""Trainium2 Bass kernel v5: 16-filter binarized 5x5 VALID conv.

x [32, 6, 512, 512] f32 -> out [32, 16, 508, 508] f32, data-parallel over
batch on 8 cores (4 images/core).

Per-core mapping (v5: single-row slabs, row-parity folded into weights):

  Row groups: 16 output rows per main group (31 groups + one 12-row tail
  per image). SBUF slot per group: [120, 512] bf16 = 6 channels x 20
  window rows, ONE image row per partition, partition index j*6+c
  (j = row-in-window). Loaded with no overlap (1.25x halo only) as two
  HWDGE DMAs per group (dim0 16 + 4 -> spreads across all 16 SDMA
  engines; a 3D DMA AP fans out by dim0 and needs dim0 <= 16).

  Matmul: K = 120 (all window rows), M = 128 = (o in 16) x (rb in 8),
  N = 508. The SAME moving AP serves both row parities: weight block
  (dx, rp) has nonzeros at K row (j, c) iff j = 2*rb + rp + dx-tap dy.
  10 accumulating matmuls per group (dx 0..4 x rp 0..1) into two PSUM
  tiles [128, 508] f32 (one bank each).

  Evacuation: DVE only (tensor_scalar_mul by per-filter alpha, f32 ->
  bf16) -- the Scalar/Sync sequencers are reserved for DMA issue, whose
  ~0.65us fixed cost per dma_start is the issue-rate wall.

  Stores: bf16, PSUM-native layout out_main [b, 128, 31*1016], one store
  per 8-group chunk (16 KB descriptors); host un-permutes + casts f32.
"""

import numpy as np
import ml_dtypes

import concourse.bass as bass
import concourse.mybir as mybir
from concourse import bacc
from concourse import tile
from concourse.bass_utils import run_bass_kernel_spmd

MAPS3 = np.array([[0, 1, 2], [1, 2, 3], [2, 3, 4], [3, 4, 5], [0, 4, 5], [0, 1, 5]])
MAPS4 = np.array(
    [
        [0, 1, 2, 3],
        [1, 2, 3, 4],
        [2, 3, 4, 5],
        [0, 3, 4, 5],
        [0, 1, 4, 5],
        [0, 1, 2, 5],
        [0, 1, 3, 4],
        [1, 2, 4, 5],
        [0, 2, 3, 5],
    ]
)

C_IN = 6
N_OUT = 16
KH = KW = 5
RG = 16  # output rows per main group
NR = 20  # window rows per main group (16 + 4 halo)
K_MAIN = C_IN * NR  # 120
NRB = 8  # row-pairs per group
M_MAIN = N_OUT * NRB  # 128
NBLK = KW * 2  # weight blocks per group: (dx, rp)
R_TAIL = 12  # output rows in the tail group (496..507)
NR_T = 16  # tail window rows (496..511)
K_TAIL = C_IN * NR_T  # 96
NRB_T = 6
M_TAIL = N_OUT * NRB_T  # 96
N_CORES = 8
NG = 8  # groups per chunk tile (chunks: 8,8,8,7)
SG = 8  # groups per staged store


def _binarize_np(w):
    w = np.asarray(w, dtype=np.float32)
    m = w - w.mean(axis=1, keepdims=True)
    c = np.clip(m, -1.0, 1.0)
    alpha = np.abs(c).mean(axis=(1, 2, 3))
    return np.sign(c).astype(np.float32), alpha.astype(np.float32)


def _filter_table(w3, w4, w6):
    s3, a3 = _binarize_np(w3)
    s4, a4 = _binarize_np(w4)
    s6, a6 = _binarize_np(w6)
    table = []
    for o in range(6):
        table.append((list(MAPS3[o]), s3[o], a3[o]))
    for o in range(9):
        table.append((list(MAPS4[o]), s4[o], a4[o]))
    table.append((list(range(6)), s6[0], a6[0]))
    return table


def _build_weight_inputs(w3, w4, w6):
    """wm [120, 10*128], wmt [96, 10*96] bf16; al [128,1], alt [96,1] f32.

    K row index = j*6 + c (j = window row, c = channel). Weight block
    b = dx*2 + rp: column m=(o,rb) nonzero at j = 2*rb + rp + dy.
    """
    table = _filter_table(w3, w4, w6)
    wm = np.zeros((K_MAIN, NBLK * M_MAIN), dtype=np.float32)
    wmt = np.zeros((K_TAIL, NBLK * M_TAIL), dtype=np.float32)
    al = np.zeros((M_MAIN, 1), dtype=np.float32)
    alt = np.zeros((M_TAIL, 1), dtype=np.float32)
    for o, (chans, sgn, alpha) in enumerate(table):
        for rb in range(NRB):
            m = o * NRB + rb
            al[m, 0] = alpha
            for dx in range(KW):
                for rp in range(2):
                    b = dx * 2 + rp
                    for ci, c in enumerate(chans):
                        for dy in range(KH):
                            j = 2 * rb + rp + dy
                            wm[j * C_IN + c, b * M_MAIN + m] = sgn[ci, dy, dx]
        for rb in range(NRB_T):
            m = o * NRB_T + rb
            alt[m, 0] = alpha
            for dx in range(KW):
                for rp in range(2):
                    b = dx * 2 + rp
                    for ci, c in enumerate(chans):
                        for dy in range(KH):
                            j = 2 * rb + rp + dy
                            wmt[j * C_IN + c, b * M_TAIL + m] = sgn[ci, dy, dx]
    return (
        wm.astype(ml_dtypes.bfloat16),
        wmt.astype(ml_dtypes.bfloat16),
        al,
        alt,
    )


def build_nc(b_per_core, h, w, num_cores=N_CORES):
    h_out, w_out = h - KH + 1, w - KW + 1
    n_groups = (h_out - R_TAIL) // RG  # 31
    assert n_groups * RG + R_TAIL == h_out
    tail_start = h_out - R_TAIL  # 496
    NN = 2 * w_out  # out elems per group-row-pair slot (1016)
    f32 = mybir.dt.float32
    bf16 = mybir.dt.bfloat16

    chunks = []
    g0 = 0
    while g0 < n_groups:
        chunks.append((g0, min(NG, n_groups - g0)))
        g0 += NG

    nc = bacc.Bacc(
        "TRN2",
        target_bir_lowering=False,
        debug=False,
        num_devices=num_cores,
    )
    x_t = nc.dram_tensor("xb", [b_per_core, C_IN, h, w], bf16, kind="ExternalInput")
    wm_t = nc.dram_tensor("wm", [K_MAIN, NBLK * M_MAIN], bf16, kind="ExternalInput")
    wmt_t = nc.dram_tensor("wmt", [K_TAIL, NBLK * M_TAIL], bf16, kind="ExternalInput")
    al_t = nc.dram_tensor("al", [M_MAIN, 1], f32, kind="ExternalInput")
    alt_t = nc.dram_tensor("alt", [M_TAIL, 1], f32, kind="ExternalInput")
    om_t = nc.dram_tensor(
        "out_main", [b_per_core, M_MAIN, n_groups * NN], bf16, kind="ExternalOutput"
    )
    ot_t = nc.dram_tensor(
        "out_tail", [b_per_core, M_TAIL, NN], bf16, kind="ExternalOutput"
    )

    with tile.TileContext(nc) as tc:
        with (
            tc.tile_pool(name="wpool", bufs=1) as wpool,
            tc.tile_pool(name="xpool", bufs=5) as xpool,
            tc.tile_pool(name="tpool", bufs=2) as tpool,
            tc.tile_pool(name="spool", bufs=3) as spool,
            tc.tile_pool(name="s2pool", bufs=2) as s2pool,
            tc.tile_pool(name="ppool", bufs=3, space="PSUM") as ppool,
            tc.tile_pool(name="p2pool", bufs=1, space="PSUM") as p2pool,
        ):
            wt = wpool.tile([K_MAIN, NBLK * M_MAIN], bf16, tag="wt")
            nc.sync.dma_start(out=wt[:], in_=wm_t[:])
            wtt = wpool.tile([K_TAIL, NBLK * M_TAIL], bf16, tag="wtt")
            nc.sync.dma_start(out=wtt[:], in_=wmt_t[:])
            at = wpool.tile([M_MAIN, 1], f32, tag="at")
            nc.sync.dma_start(out=at[:], in_=al_t[:])
            att = wpool.tile([M_TAIL, 1], f32, tag="att")
            nc.sync.dma_start(out=att[:], in_=alt_t[:])

            def load_chunk(b, ci):
                g0, ng = chunks[ci]
                xt = xpool.tile(
                    [K_MAIN, NG * w], bf16, tag="xt", name=f"xt_{b}_{ci}"
                )
                # one row per partition, partition index j*6+c; 3D DMA AP
                # fans out by dim0 (must be <=16): split j 20 -> 16 + 4
                for gl in range(ng):
                    base = b * C_IN * h * w + RG * (g0 + gl) * w
                    eng = nc.sync if gl % 2 == 0 else nc.scalar
                    src_a = bass.AP(
                        x_t, base, [[w, 16], [h * w, C_IN], [1, w]]
                    )
                    eng.dma_start(
                        out=xt[0 : 16 * C_IN, gl * w : gl * w + w], in_=src_a
                    )
                    src_b = bass.AP(
                        x_t, base + 16 * w, [[w, 4], [h * w, C_IN], [1, w]]
                    )
                    eng.dma_start(
                        out=xt[16 * C_IN : K_MAIN, gl * w : gl * w + w],
                        in_=src_b,
                    )
                return xt

            def load_tail(b):
                xt2 = tpool.tile([K_TAIL, w], bf16, tag="xtt", name=f"xtt_{b}")
                src = bass.AP(
                    x_t,
                    b * C_IN * h * w + tail_start * w,
                    [[w, NR_T], [h * w, C_IN], [1, w]],
                )
                nc.sync.dma_start(out=xt2[:], in_=src)
                return xt2

            def do_group(xt, gl, stg, soff):
                pss = [
                    ppool.tile([M_MAIN, w_out], f32, tag=f"ps{rp}", name=f"ps{rp}")
                    for rp in range(2)
                ]
                for dx in range(KW):
                    rhs = bass.AP(
                        xt[:].tensor,
                        gl * w + dx,
                        [[NG * w, K_MAIN], [1, w_out]],
                    )
                    for rp in range(2):
                        blk = dx * 2 + rp
                        nc.tensor.matmul(
                            pss[rp][:],
                            wt[:, blk * M_MAIN : (blk + 1) * M_MAIN],
                            rhs,
                            start=(dx == 0),
                            stop=(dx == KW - 1),
                        )
                for rp in range(2):
                    dst = stg[:, soff + rp * w_out : soff + (rp + 1) * w_out]
                    nc.vector.tensor_scalar_mul(dst, pss[rp][:], at[:])

            def store_stage(b, gs, stg, ng_st):
                dst = bass.AP(
                    om_t,
                    b * M_MAIN * n_groups * NN + gs * NN,
                    [[n_groups * NN, M_MAIN], [1, ng_st * NN]],
                )
                nc.scalar.dma_start(out=dst, in_=stg[:, : ng_st * NN])

            def do_tail(b, xt2):
                pss = [
                    p2pool.tile([M_TAIL, w_out], f32, tag=f"pst{rp}", name=f"pst{rp}")
                    for rp in range(2)
                ]
                for dx in range(KW):
                    rhs = bass.AP(
                        xt2[:].tensor,
                        dx,
                        [[w, K_TAIL], [1, w_out]],
                    )
                    for rp in range(2):
                        blk = dx * 2 + rp
                        nc.tensor.matmul(
                            pss[rp][:],
                            wtt[:, blk * M_TAIL : (blk + 1) * M_TAIL],
                            rhs,
                            start=(dx == 0),
                            stop=(dx == KW - 1),
                        )
                st = s2pool.tile([M_TAIL, NN], bf16, tag="st2")
                for rp in range(2):
                    nc.vector.tensor_scalar_mul(
                        st[:, rp * w_out : (rp + 1) * w_out], pss[rp][:], att[:]
                    )
                dst = bass.AP(
                    ot_t, b * M_TAIL * NN, [[NN, M_TAIL], [1, NN]]
                )
                nc.scalar.dma_start(out=dst, in_=st[:])

            units = [(b, ci) for b in range(b_per_core) for ci in range(len(chunks))]
            LOOKAHEAD = 4
            xtiles = {}
            ttiles = {}
            for u in range(min(LOOKAHEAD, len(units))):
                b, ci = units[u]
                xtiles[(b, ci)] = load_chunk(b, ci)
                if ci == 0:
                    ttiles[b] = load_tail(b)

            for u, (b, ci) in enumerate(units):
                ul = u + LOOKAHEAD
                if ul < len(units):
                    bl, cl = units[ul]
                    xtiles[(bl, cl)] = load_chunk(bl, cl)
                    if cl == 0:
                        ttiles[bl] = load_tail(bl)
                xt = xtiles.pop((b, ci))
                g0, ng = chunks[ci]
                gl = 0
                while gl < ng:
                    ns = min(SG, ng - gl)
                    stg = spool.tile(
                        [M_MAIN, SG * NN], bf16, tag="stg",
                        name=f"stg_{b}_{ci}_{gl}",
                    )
                    for gg in range(ns):
                        do_group(xt, gl + gg, stg, gg * NN)
                    store_stage(b, g0 + gl, stg, ns)
                    gl += ns
                if ci == len(chunks) - 1:
                    do_tail(b, ttiles.pop(b))

    nc.compile()
    return nc


_NC_CACHE = {}


def _get_nc(b_per_core, h, w):
    key = (b_per_core, h, w)
    if key not in _NC_CACHE:
        _NC_CACHE[key] = build_nc(b_per_core, h, w)
    return _NC_CACHE[key]


def _prep_inputs(x, w3, w4, w6):
    b = x.shape[0]
    assert b % N_CORES == 0
    bpc = b // N_CORES
    wm, wmt, al, alt = _build_weight_inputs(w3, w4, w6)
    xb = np.ascontiguousarray(x).astype(ml_dtypes.bfloat16)
    in_maps = [
        {
            "xb": np.ascontiguousarray(xb[i * bpc : (i + 1) * bpc]),
            "wm": wm,
            "wmt": wmt,
            "al": al,
            "alt": alt,
        }
        for i in range(N_CORES)
    ]
    return bpc, in_maps


def _unpermute(om, ot, bpc, h_out, w_out):
    """om [bpc, 128, 31*1016], ot [bpc, 96, 1016] -> [bpc, 16, 508, 508]."""
    n_groups = (h_out - R_TAIL) // RG
    out = np.empty((bpc, N_OUT, h_out, w_out), dtype=np.float32)
    m = om.reshape(bpc, N_OUT, NRB, n_groups, 2, w_out)
    out[:, :, : n_groups * RG] = m.transpose(0, 1, 3, 2, 4, 5).reshape(
        bpc, N_OUT, n_groups * RG, w_out
    )
    t = ot.reshape(bpc, N_OUT, NRB_T * 2, w_out)
    out[:, :, n_groups * RG :] = t
    return out


def run(x, w3, w4, w6, trace=False, **kw):
    b, c, h, w = x.shape
    h_out, w_out = h - 4, w - 4
    bpc, in_maps = _prep_inputs(x, w3, w4, w6)
    nc = _get_nc(bpc, h, w)
    res = run_bass_kernel_spmd(
        nc, in_maps, list(range(N_CORES)), trace=trace, **kw
    )
    outs = [
        _unpermute(
            np.asarray(r["out_main"], dtype=np.float32),
            np.asarray(r["out_tail"], dtype=np.float32),
            bpc, h_out, w_out,
        )
        for r in res.results
    ]
    return np.concatenate(outs, axis=0), res


def kernel(x, w3, w4, w6):
    out, _ = run(x, w3, w4, w6, trace=False)
    return out


# revision 17
# speedup vs baseline: 4.2089x; 1.0014x over previous
"""Trainium2 Bass kernel v5: 16-filter binarized 5x5 VALID conv.

x [32, 6, 512, 512] f32 -> out [32, 16, 508, 508] f32, data-parallel over
batch on 8 cores (4 images/core).

Per-core mapping (v5: single-row slabs, row-parity folded into weights):

  Row groups: 16 output rows per main group (31 groups + one 12-row tail
  per image). SBUF slot per group: [120, 512] bf16 = 6 channels x 20
  window rows, ONE image row per partition, partition index j*6+c
  (j = row-in-window). Loaded with no overlap (1.25x halo only) as two
  HWDGE DMAs per group (dim0 16 + 4 -> spreads across all 16 SDMA
  engines; a 3D DMA AP fans out by dim0 and needs dim0 <= 16).

  Matmul: K = 120 (all window rows), M = 128 = (o in 16) x (rb in 8),
  N = 508. The SAME moving AP serves both row parities: weight block
  (dx, rp) has nonzeros at K row (j, c) iff j = 2*rb + rp + dx-tap dy.
  10 accumulating matmuls per group (dx 0..4 x rp 0..1) into two PSUM
  tiles [128, 508] f32 (one bank each).

  Evacuation: DVE only (tensor_scalar_mul by per-filter alpha, f32 ->
  bf16) -- the Scalar/Sync sequencers are reserved for DMA issue, whose
  ~0.65us fixed cost per dma_start is the issue-rate wall.

  Stores: bf16, PSUM-native layout out_main [b, 128, 31*1016], one store
  per 8-group chunk (16 KB descriptors); host un-permutes + casts f32.
"""

import numpy as np
import ml_dtypes

import concourse.bass as bass
import concourse.mybir as mybir
from concourse import bacc
from concourse import tile
from concourse.bass_utils import run_bass_kernel_spmd

MAPS3 = np.array([[0, 1, 2], [1, 2, 3], [2, 3, 4], [3, 4, 5], [0, 4, 5], [0, 1, 5]])
MAPS4 = np.array(
    [
        [0, 1, 2, 3],
        [1, 2, 3, 4],
        [2, 3, 4, 5],
        [0, 3, 4, 5],
        [0, 1, 4, 5],
        [0, 1, 2, 5],
        [0, 1, 3, 4],
        [1, 2, 4, 5],
        [0, 2, 3, 5],
    ]
)

C_IN = 6
N_OUT = 16
KH = KW = 5
RG = 16  # output rows per main group
NR = 20  # window rows per main group (16 + 4 halo)
K_MAIN = C_IN * NR  # 120
NRB = 8  # row-pairs per group
M_MAIN = N_OUT * NRB  # 128
NBLK = KW * 2  # weight blocks per group: (dx, rp)
R_TAIL = 12  # output rows in the tail group (496..507)
NR_T = 16  # tail window rows (496..511)
K_TAIL = C_IN * NR_T  # 96
NRB_T = 6
M_TAIL = N_OUT * NRB_T  # 96
N_CORES = 8
NG = 8  # groups per chunk tile (chunks: 8,8,8,7)
SG = 4  # groups per staged store


def _binarize_np(w):
    w = np.asarray(w, dtype=np.float32)
    m = w - w.mean(axis=1, keepdims=True)
    c = np.clip(m, -1.0, 1.0)
    alpha = np.abs(c).mean(axis=(1, 2, 3))
    return np.sign(c).astype(np.float32), alpha.astype(np.float32)


def _filter_table(w3, w4, w6):
    s3, a3 = _binarize_np(w3)
    s4, a4 = _binarize_np(w4)
    s6, a6 = _binarize_np(w6)
    table = []
    for o in range(6):
        table.append((list(MAPS3[o]), s3[o], a3[o]))
    for o in range(9):
        table.append((list(MAPS4[o]), s4[o], a4[o]))
    table.append((list(range(6)), s6[0], a6[0]))
    return table


def _build_weight_inputs(w3, w4, w6):
    """wm [120, 10*128], wmt [96, 10*96] bf16; al [128,1], alt [96,1] f32.

    K row index = j*6 + c (j = window row, c = channel). Weight block
    b = dx*2 + rp: column m=(o,rb) nonzero at j = 2*rb + rp + dy.
    """
    table = _filter_table(w3, w4, w6)
    wm = np.zeros((K_MAIN, NBLK * M_MAIN), dtype=np.float32)
    wmt = np.zeros((K_TAIL, NBLK * M_TAIL), dtype=np.float32)
    al = np.zeros((M_MAIN, 1), dtype=np.float32)
    alt = np.zeros((M_TAIL, 1), dtype=np.float32)
    for o, (chans, sgn, alpha) in enumerate(table):
        for rb in range(NRB):
            m = o * NRB + rb
            al[m, 0] = alpha
            for dx in range(KW):
                for rp in range(2):
                    b = dx * 2 + rp
                    for ci, c in enumerate(chans):
                        for dy in range(KH):
                            j = 2 * rb + rp + dy
                            wm[j * C_IN + c, b * M_MAIN + m] = sgn[ci, dy, dx]
        for rb in range(NRB_T):
            m = o * NRB_T + rb
            alt[m, 0] = alpha
            for dx in range(KW):
                for rp in range(2):
                    b = dx * 2 + rp
                    for ci, c in enumerate(chans):
                        for dy in range(KH):
                            j = 2 * rb + rp + dy
                            wmt[j * C_IN + c, b * M_TAIL + m] = sgn[ci, dy, dx]
    return (
        wm.astype(ml_dtypes.bfloat16),
        wmt.astype(ml_dtypes.bfloat16),
        al,
        alt,
    )


def build_nc(b_per_core, h, w, num_cores=N_CORES):
    h_out, w_out = h - KH + 1, w - KW + 1
    n_groups = (h_out - R_TAIL) // RG  # 31
    assert n_groups * RG + R_TAIL == h_out
    tail_start = h_out - R_TAIL  # 496
    NN = 2 * w_out  # out elems per group-row-pair slot (1016)
    f32 = mybir.dt.float32
    bf16 = mybir.dt.bfloat16

    chunks = []
    g0 = 0
    while g0 < n_groups:
        chunks.append((g0, min(NG, n_groups - g0)))
        g0 += NG

    nc = bacc.Bacc(
        "TRN2",
        target_bir_lowering=False,
        debug=False,
        num_devices=num_cores,
    )
    x_t = nc.dram_tensor("xb", [b_per_core, C_IN, h, w], bf16, kind="ExternalInput")
    wm_t = nc.dram_tensor("wm", [K_MAIN, NBLK * M_MAIN], bf16, kind="ExternalInput")
    wmt_t = nc.dram_tensor("wmt", [K_TAIL, NBLK * M_TAIL], bf16, kind="ExternalInput")
    al_t = nc.dram_tensor("al", [M_MAIN, 1], f32, kind="ExternalInput")
    alt_t = nc.dram_tensor("alt", [M_TAIL, 1], f32, kind="ExternalInput")
    om_t = nc.dram_tensor(
        "out_main", [b_per_core, M_MAIN, n_groups * NN], bf16, kind="ExternalOutput"
    )
    ot_t = nc.dram_tensor(
        "out_tail", [b_per_core, M_TAIL, NN], bf16, kind="ExternalOutput"
    )

    with tile.TileContext(nc) as tc:
        with (
            tc.tile_pool(name="wpool", bufs=1) as wpool,
            tc.tile_pool(name="xpool", bufs=5) as xpool,
            tc.tile_pool(name="tpool", bufs=2) as tpool,
            tc.tile_pool(name="spool", bufs=3) as spool,
            tc.tile_pool(name="s2pool", bufs=2) as s2pool,
            tc.tile_pool(name="ppool", bufs=3, space="PSUM") as ppool,
            tc.tile_pool(name="p2pool", bufs=1, space="PSUM") as p2pool,
        ):
            # weight loads in <=16-partition chunks so each dma fans out
            # across SDMA engines (dim0 <= 16) instead of one engine
            WM_F = NBLK * M_MAIN  # 1280
            wt = wpool.tile([K_MAIN, WM_F], bf16, tag="wt")
            for i in range(8):
                eng = nc.sync if i % 2 == 0 else nc.scalar
                eng.dma_start(
                    out=wt[i * 15 : (i + 1) * 15, :],
                    in_=bass.AP(wm_t, i * 15 * WM_F, [[WM_F, 15], [1, WM_F]]),
                )
            WT_F = NBLK * M_TAIL  # 960
            wtt = wpool.tile([K_TAIL, WT_F], bf16, tag="wtt")
            for i in range(6):
                eng = nc.sync if i % 2 == 0 else nc.scalar
                eng.dma_start(
                    out=wtt[i * 16 : (i + 1) * 16, :],
                    in_=bass.AP(wmt_t, i * 16 * WT_F, [[WT_F, 16], [1, WT_F]]),
                )
            at = wpool.tile([M_MAIN, 1], f32, tag="at")
            nc.sync.dma_start(out=at[:], in_=al_t[:])
            att = wpool.tile([M_TAIL, 1], f32, tag="att")
            nc.sync.dma_start(out=att[:], in_=alt_t[:])

            def load_chunk(b, ci):
                g0, ng = chunks[ci]
                xt = xpool.tile(
                    [K_MAIN, NG * w], bf16, tag="xt", name=f"xt_{b}_{ci}"
                )
                # one row per partition, partition index j*6+c; 3D DMA AP
                # fans out by dim0 (must be <=16): split j 20 -> 16 + 4
                for gl in range(ng):
                    base = b * C_IN * h * w + RG * (g0 + gl) * w
                    eng = nc.sync if gl % 2 == 0 else nc.scalar
                    src_a = bass.AP(
                        x_t, base, [[w, 16], [h * w, C_IN], [1, w]]
                    )
                    eng.dma_start(
                        out=xt[0 : 16 * C_IN, gl * w : gl * w + w], in_=src_a
                    )
                    src_b = bass.AP(
                        x_t, base + 16 * w, [[w, 4], [h * w, C_IN], [1, w]]
                    )
                    eng.dma_start(
                        out=xt[16 * C_IN : K_MAIN, gl * w : gl * w + w],
                        in_=src_b,
                    )
                return xt

            def load_tail(b):
                xt2 = tpool.tile([K_TAIL, w], bf16, tag="xtt", name=f"xtt_{b}")
                src = bass.AP(
                    x_t,
                    b * C_IN * h * w + tail_start * w,
                    [[w, NR_T], [h * w, C_IN], [1, w]],
                )
                nc.sync.dma_start(out=xt2[:], in_=src)
                return xt2

            def do_group(xt, gl, stg, soff):
                pss = [
                    ppool.tile([M_MAIN, w_out], f32, tag=f"ps{rp}", name=f"ps{rp}")
                    for rp in range(2)
                ]
                for dx in range(KW):
                    rhs = bass.AP(
                        xt[:].tensor,
                        gl * w + dx,
                        [[NG * w, K_MAIN], [1, w_out]],
                    )
                    for rp in range(2):
                        blk = dx * 2 + rp
                        nc.tensor.matmul(
                            pss[rp][:],
                            wt[:, blk * M_MAIN : (blk + 1) * M_MAIN],
                            rhs,
                            start=(dx == 0),
                            stop=(dx == KW - 1),
                        )
                for rp in range(2):
                    dst = stg[:, soff + rp * w_out : soff + (rp + 1) * w_out]
                    nc.vector.tensor_scalar_mul(dst, pss[rp][:], at[:])

            def store_stage(b, gs, stg, ng_st):
                dst = bass.AP(
                    om_t,
                    b * M_MAIN * n_groups * NN + gs * NN,
                    [[n_groups * NN, M_MAIN], [1, ng_st * NN]],
                )
                nc.scalar.dma_start(out=dst, in_=stg[:, : ng_st * NN])

            def do_tail(b, xt2):
                pss = [
                    p2pool.tile([M_TAIL, w_out], f32, tag=f"pst{rp}", name=f"pst{rp}")
                    for rp in range(2)
                ]
                for dx in range(KW):
                    rhs = bass.AP(
                        xt2[:].tensor,
                        dx,
                        [[w, K_TAIL], [1, w_out]],
                    )
                    for rp in range(2):
                        blk = dx * 2 + rp
                        nc.tensor.matmul(
                            pss[rp][:],
                            wtt[:, blk * M_TAIL : (blk + 1) * M_TAIL],
                            rhs,
                            start=(dx == 0),
                            stop=(dx == KW - 1),
                        )
                st = s2pool.tile([M_TAIL, NN], bf16, tag="st2")
                for rp in range(2):
                    nc.vector.tensor_scalar_mul(
                        st[:, rp * w_out : (rp + 1) * w_out], pss[rp][:], att[:]
                    )
                dst = bass.AP(
                    ot_t, b * M_TAIL * NN, [[NN, M_TAIL], [1, NN]]
                )
                nc.scalar.dma_start(out=dst, in_=st[:])

            units = [(b, ci) for b in range(b_per_core) for ci in range(len(chunks))]
            LOOKAHEAD = 4
            xtiles = {}
            ttiles = {}
            for u in range(min(LOOKAHEAD, len(units))):
                b, ci = units[u]
                xtiles[(b, ci)] = load_chunk(b, ci)
                if ci == 0:
                    ttiles[b] = load_tail(b)

            for u, (b, ci) in enumerate(units):
                ul = u + LOOKAHEAD
                if ul < len(units):
                    bl, cl = units[ul]
                    xtiles[(bl, cl)] = load_chunk(bl, cl)
                    if cl == 0:
                        ttiles[bl] = load_tail(bl)
                xt = xtiles.pop((b, ci))
                g0, ng = chunks[ci]
                gl = 0
                while gl < ng:
                    ns = min(SG, ng - gl)
                    stg = spool.tile(
                        [M_MAIN, SG * NN], bf16, tag="stg",
                        name=f"stg_{b}_{ci}_{gl}",
                    )
                    for gg in range(ns):
                        do_group(xt, gl + gg, stg, gg * NN)
                    store_stage(b, g0 + gl, stg, ns)
                    gl += ns
                if ci == 0:
                    do_tail(b, ttiles.pop(b))

    nc.compile()
    return nc


_NC_CACHE = {}


def _get_nc(b_per_core, h, w):
    key = (b_per_core, h, w)
    if key not in _NC_CACHE:
        _NC_CACHE[key] = build_nc(b_per_core, h, w)
    return _NC_CACHE[key]


def _prep_inputs(x, w3, w4, w6):
    b = x.shape[0]
    assert b % N_CORES == 0
    bpc = b // N_CORES
    wm, wmt, al, alt = _build_weight_inputs(w3, w4, w6)
    xb = np.ascontiguousarray(x).astype(ml_dtypes.bfloat16)
    in_maps = [
        {
            "xb": np.ascontiguousarray(xb[i * bpc : (i + 1) * bpc]),
            "wm": wm,
            "wmt": wmt,
            "al": al,
            "alt": alt,
        }
        for i in range(N_CORES)
    ]
    return bpc, in_maps


def _unpermute(om, ot, bpc, h_out, w_out):
    """om [bpc, 128, 31*1016], ot [bpc, 96, 1016] -> [bpc, 16, 508, 508]."""
    n_groups = (h_out - R_TAIL) // RG
    out = np.empty((bpc, N_OUT, h_out, w_out), dtype=np.float32)
    m = om.reshape(bpc, N_OUT, NRB, n_groups, 2, w_out)
    out[:, :, : n_groups * RG] = m.transpose(0, 1, 3, 2, 4, 5).reshape(
        bpc, N_OUT, n_groups * RG, w_out
    )
    t = ot.reshape(bpc, N_OUT, NRB_T * 2, w_out)
    out[:, :, n_groups * RG :] = t
    return out


def run(x, w3, w4, w6, trace=False, **kw):
    b, c, h, w = x.shape
    h_out, w_out = h - 4, w - 4
    bpc, in_maps = _prep_inputs(x, w3, w4, w6)
    nc = _get_nc(bpc, h, w)
    res = run_bass_kernel_spmd(
        nc, in_maps, list(range(N_CORES)), trace=trace, **kw
    )
    outs = [
        _unpermute(
            np.asarray(r["out_main"], dtype=np.float32),
            np.asarray(r["out_tail"], dtype=np.float32),
            bpc, h_out, w_out,
        )
        for r in res.results
    ]
    return np.concatenate(outs, axis=0), res


def kernel(x, w3, w4, w6):
    out, _ = run(x, w3, w4, w6, trace=False)
    return out
